# revision 8
# baseline (speedup 1.0000x reference)
"""GroupTopk Trainium2 kernel.

x: [16, 512, 64, 64] f32. Per pixel, per group of 32 channels: top-4 values
(descending), grouped 1x1 conv [4 -> 32] with per-group weight w[g, o, k],
residual add. out = x + enhanced.

Strategy (8 cores, data-parallel over N, 2 images/core):
 - DMA x in channel-major (contiguous), tile = 512 pixels x 512 channels.
 - PE transposes x into PSUM pixel-major [128 px, 512 ch].
 - Bitonic merge network (tensor_tensor max/min on strided views) computes
   sorted top-4 per (pixel, group); max-ops on DVE, min-ops on GpSimd(Pool).
 - PE transposes top-4 block [128, 64] -> [64, 128]; fp32r matmul with the
   block-diagonal conv weight produces enhanced channel-major in PSUM.
 - DVE adds x (channel-major, still in SBUF) to PSUM result -> out tile.
 - DMA out channel-major (contiguous).
"""

import numpy as np
from contextlib import ExitStack

import concourse.bacc as bacc
import concourse.bass as bass
import concourse.mybir as mybir
import concourse.tile as tile
from concourse.bass_utils import run_bass_kernel_spmd

F32 = mybir.dt.float32
F32R = mybir.dt.float32r

N, C, H, W = 16, 512, 64, 64
HW = H * W            # 4096
G, GS, K = 16, 32, 4  # groups, group size, topk
NCORES = 8
IMGS = N // NCORES    # images per core
PB = 4                # 128-pixel blocks per batch
BPX = PB * 128        # 512 pixels per batch
NBATCH = IMGS * HW // BPX

MAX = mybir.AluOpType.max
MIN = mybir.AluOpType.min

# engine per sort op: "v" = DVE, "g" = gpsimd/Pool  (tuned from traces)
# gpsimd/Pool cannot run TensorTensor in this compiler -> everything on DVE
ENG = {k: "v" for k in (
    "A1", "A2", "B1", "B2", "B3", "B4", "B5", "B6",
    "C1", "C2", "C2b", "C3", "C3b", "D1", "D2", "D2b", "D3", "D3b",
    "E1", "E2", "E2b", "E3", "E3b")}


def _v(t, off, dims):
    """Strided view of a [128, F] (or [P, F]) tile: keep partition dim, set free dims."""
    b = t[:]
    return bass.AP(
        tensor=b.tensor,
        offset=b.offset + off,
        ap=[list(b.ap[0])] + [list(d) for d in dims],
    )


def _build_nc():
    nc = bacc.Bacc("TRN2", target_bir_lowering=False, debug=False)
    x_d = nc.dram_tensor("x", [IMGS, C, HW], F32, kind="ExternalInput").ap()
    wbd_d = nc.dram_tensor("wbd", [64, C], F32, kind="ExternalInput").ap()
    id_d = nc.dram_tensor("ident", [128, 128], F32, kind="ExternalInput").ap()
    out_d = nc.dram_tensor("out", [IMGS, C, HW], F32, kind="ExternalOutput").ap()

    with tile.TileContext(nc) as tc:
        with ExitStack() as ctx:
            _emit(ctx, tc, nc, out_d, x_d, wbd_d, id_d)
    nc.compile()
    return nc


def _emit(ctx, tc, nc, out_d, x_d, wbd_d, id_d):
    consts = ctx.enter_context(tc.tile_pool(name="consts", bufs=1))
    xpool = ctx.enter_context(tc.tile_pool(name="xt", bufs=2))
    opool = ctx.enter_context(tc.tile_pool(name="osb", bufs=2))
    wpool = ctx.enter_context(tc.tile_pool(name="work", bufs=2))
    psx_pool = ctx.enter_context(tc.tile_pool(name="psx", bufs=1, space="PSUM"))
    pst_pool = ctx.enter_context(tc.tile_pool(name="pst", bufs=2, space="PSUM"))
    psc_pool = ctx.enter_context(tc.tile_pool(name="psc", bufs=2, space="PSUM"))

    wbd = consts.tile([64, C], F32)
    nc.sync.dma_start(wbd[:], wbd_d[:])
    ident = consts.tile([128, 128], F32)
    nc.sync.dma_start(ident[:], id_d[:])

    def tt(name, out, a, b):
        eng = nc.vector if ENG[name] == "v" else nc.gpsimd
        is_max = name in ("A1", "B1", "B4", "B5", "C1", "C2", "C3", "D1", "D2", "D3", "E1", "E2", "E3")
        eng.tensor_tensor(out, a, b, MAX if is_max else MIN)

    def stage_load(bi):
        img = bi // (HW // BPX)
        px0 = (bi % (HW // BPX)) * BPX

        # ---- load x channel-major: xt[cc, cb*512 + px] = x[img, cb*128+cc, px0+px]
        xt = xpool.tile([128, PB * 512], F32)
        dram = bass.AP(tensor=x_d.tensor, offset=x_d[:].offset + img * C * HW + px0,
                       ap=[[HW, 128], [128 * HW, 4], [1, BPX]])
        sb = _v(xt, 0, [[BPX, 4], [1, BPX]])
        nc.sync.dma_start(sb, dram)

        # ---- PE transpose -> psX[px_part, pb*512 + ch] (pixel-major)
        psX = psx_pool.tile([128, PB * 512], F32)
        for pb in range(PB):
            for cb in range(4):
                src = _v(xt, cb * BPX + pb * 128, [[1, 128]])  # [128cc, 128px]
                dst = _v(psX, pb * 512 + cb * 128, [[1, 128]])  # [128px, 128ch]
                nc.tensor.transpose(dst, src, ident[:])

        # ---- evacuate pixel-major x to SBUF (Act; TT can read at most one PSUM operand)
        Xp = wpool.tile([128, PB * 512], F32)
        nc.scalar.copy(Xp[:], psX[:])
        return xt, Xp

    def stage_compute(bi, xt, Xp):
        img = bi // (HW // BPX)
        px0 = (bi % (HW // BPX)) * BPX
        # ---- sort network ----
        # views are [pg(=pb*16+g): 64, ...] merged; Xp ch = g*32 + j
        PH = wpool.tile([128, 1024], F32)
        PL = wpool.tile([128, 1024], F32)
        Xe = _v(Xp, 0, [[32, 64], [2, 16]])
        Xo = _v(Xp, 1, [[32, 64], [2, 16]])
        tt("A1", _v(PH, 0, [[16, 64], [1, 16]]), Xe, Xo)
        tt("A2", _v(PL, 0, [[16, 64], [1, 16]]), Xe, Xo)

        Y = wpool.tile([128, 2048], F32)
        M1 = wpool.tile([128, 512], F32)
        M2 = wpool.tile([128, 512], F32)
        PHe = _v(PH, 0, [[16, 64], [2, 8]])
        PHo = _v(PH, 1, [[16, 64], [2, 8]])
        PLe = _v(PL, 0, [[16, 64], [2, 8]])
        PLo = _v(PL, 1, [[16, 64], [2, 8]])
        m1v = _v(M1, 0, [[8, 64], [1, 8]])
        m2v = _v(M2, 0, [[8, 64], [1, 8]])
        tt("B1", _v(Y, 0, [[32, 64], [4, 8]]), PHe, PHo)
        tt("B2", m1v, PHe, PHo)
        tt("B3", _v(Y, 3, [[32, 64], [4, 8]]), PLe, PLo)
        tt("B4", m2v, PLe, PLo)
        tt("B5", _v(Y, 1, [[32, 64], [4, 8]]), m1v, m2v)
        tt("B6", _v(Y, 2, [[32, 64], [4, 8]]), m1v, m2v)

        Zb = wpool.tile([128, 1024], F32)
        Zs = wpool.tile([128, 1024], F32)
        Z = wpool.tile([128, 1024], F32)
        tt("C1", _v(Zb, 0, [[16, 64], [4, 4], [1, 4]]),
           _v(Y, 0, [[32, 64], [8, 4], [1, 4]]),
           _v(Y, 7, [[32, 64], [8, 4], [-1, 4]]))
        tt("C2", _v(Zs, 0, [[16, 64], [4, 4], [1, 2]]),
           _v(Zb, 0, [[16, 64], [4, 4], [1, 2]]),
           _v(Zb, 2, [[16, 64], [4, 4], [1, 2]]))
        tt("C2b", _v(Zs, 2, [[16, 64], [4, 4], [1, 2]]),
           _v(Zb, 0, [[16, 64], [4, 4], [1, 2]]),
           _v(Zb, 2, [[16, 64], [4, 4], [1, 2]]))
        tt("C3", _v(Z, 0, [[16, 64], [4, 4], [2, 2]]),
           _v(Zs, 0, [[16, 64], [4, 4], [2, 2]]),
           _v(Zs, 1, [[16, 64], [4, 4], [2, 2]]))
        tt("C3b", _v(Z, 1, [[16, 64], [4, 4], [2, 2]]),
           _v(Zs, 0, [[16, 64], [4, 4], [2, 2]]),
           _v(Zs, 1, [[16, 64], [4, 4], [2, 2]]))

        Vb = wpool.tile([128, 512], F32)
        Vs = wpool.tile([128, 512], F32)
        Vt = wpool.tile([128, 512], F32)
        tt("D1", _v(Vb, 0, [[8, 64], [4, 2], [1, 4]]),
           _v(Z, 0, [[16, 64], [8, 2], [1, 4]]),
           _v(Z, 7, [[16, 64], [8, 2], [-1, 4]]))
        tt("D2", _v(Vs, 0, [[8, 64], [4, 2], [1, 2]]),
           _v(Vb, 0, [[8, 64], [4, 2], [1, 2]]),
           _v(Vb, 2, [[8, 64], [4, 2], [1, 2]]))
        tt("D2b", _v(Vs, 2, [[8, 64], [4, 2], [1, 2]]),
           _v(Vb, 0, [[8, 64], [4, 2], [1, 2]]),
           _v(Vb, 2, [[8, 64], [4, 2], [1, 2]]))
        tt("D3", _v(Vt, 0, [[8, 64], [4, 2], [2, 2]]),
           _v(Vs, 0, [[8, 64], [4, 2], [2, 2]]),
           _v(Vs, 1, [[8, 64], [4, 2], [2, 2]]))
        tt("D3b", _v(Vt, 1, [[8, 64], [4, 2], [2, 2]]),
           _v(Vs, 0, [[8, 64], [4, 2], [2, 2]]),
           _v(Vs, 1, [[8, 64], [4, 2], [2, 2]]))

        Mb = wpool.tile([128, 256], F32)
        Ms = wpool.tile([128, 256], F32)
        mm = wpool.tile([128, 256], F32)
        tt("E1", _v(Mb, 0, [[4, 64], [1, 4]]),
           _v(Vt, 0, [[8, 64], [1, 4]]),
           _v(Vt, 7, [[8, 64], [-1, 4]]))
        tt("E2", _v(Ms, 0, [[4, 64], [1, 2]]),
           _v(Mb, 0, [[4, 64], [1, 2]]),
           _v(Mb, 2, [[4, 64], [1, 2]]))
        tt("E2b", _v(Ms, 2, [[4, 64], [1, 2]]),
           _v(Mb, 0, [[4, 64], [1, 2]]),
           _v(Mb, 2, [[4, 64], [1, 2]]))
        tt("E3", _v(mm, 0, [[4, 64], [2, 2]]),
           _v(Ms, 0, [[4, 64], [2, 2]]),
           _v(Ms, 1, [[4, 64], [2, 2]]))
        tt("E3b", _v(mm, 1, [[4, 64], [2, 2]]),
           _v(Ms, 0, [[4, 64], [2, 2]]),
           _v(Ms, 1, [[4, 64], [2, 2]]))

        # ---- conv: enhanced channel-major, then + x, store ----
        mT = wpool.tile([64, 512], F32)
        for pb in range(PB):
            pst = pst_pool.tile([64, 128], F32)
            nc.tensor.transpose(pst[:], _v(mm, pb * 64, [[1, 64]]), ident[:])
            nc.scalar.copy(_v(mT, pb * 128, [[1, 128]]), pst[:])

        osb = opool.tile([128, PB * 512], F32)
        for cb in range(4):
            psc = psc_pool.tile([128, 512], F32)
            nc.tensor.matmul(
                psc[:],
                _v(wbd, cb * 128, [[1, 128]]),
                mT[:],
            )
            nc.vector.scalar_tensor_tensor(
                _v(osb, cb * BPX, [[1, BPX]]),
                psc[:],
                0.0,
                _v(xt, cb * BPX, [[1, BPX]]),
                mybir.AluOpType.add,
                mybir.AluOpType.add,
            )

        dram_o = bass.AP(tensor=out_d.tensor, offset=out_d[:].offset + img * C * HW + px0,
                         ap=[[HW, 128], [128 * HW, 4], [1, BPX]])
        nc.scalar.dma_start(dram_o, _v(osb, 0, [[BPX, 4], [1, BPX]]))

    prev = stage_load(0)
    for bi in range(NBATCH):
        nxt = stage_load(bi + 1) if bi + 1 < NBATCH else None
        stage_compute(bi, *prev)
        prev = nxt


_NC_CACHE = None


def _get_nc():
    global _NC_CACHE
    if _NC_CACHE is None:
        _NC_CACHE = _build_nc()
    return _NC_CACHE


def _host_wbd(w):
    wbd = np.zeros((64, C), dtype=np.float32)
    wt = np.transpose(w.astype(np.float32), (0, 2, 1))  # [g, k, o]
    for g in range(G):
        wbd[g * 4:(g + 1) * 4, g * 32:(g + 1) * 32] = wt[g]
    return wbd


def run(x, w, trace=False):
    nc = _get_nc()
    xr = np.ascontiguousarray(x.astype(np.float32).reshape(N, C, HW))
    wbd = _host_wbd(w)
    ident = np.eye(128, dtype=np.float32)
    in_maps = [
        {"x": xr[c * IMGS:(c + 1) * IMGS], "wbd": wbd, "ident": ident}
        for c in range(NCORES)
    ]
    res = run_bass_kernel_spmd(nc, in_maps, core_ids=list(range(NCORES)), trace=trace)
    out = np.stack([r["out"] for r in res.results])  # [8, IMGS, C, HW]
    out = out.reshape(N, C, H, W)
    return out, res


def kernel(x, w):
    out, _ = run(x, w, trace=False)
    return out.astype(np.float32)
